# revision 6
# baseline (speedup 1.0000x reference)
"""Bahdanau attention forward on 8 Trainium2 NeuronCores (data-parallel).

Layout: value rows flattened to r = b*W + w, placed at partition p = r % 120
(cols = r // 120), padded to 128 partitions (8 dead rows, masked out).  Each
column j holds the 120 rows of a 6-batch window (b = 6j + p//20, w = p%20).

Per-core pipeline, G=64 columns per block:
  1. fp16 loads: value tile VT and a host-replicated query tensor QT
     (qin[r] = (q*W1)[b(r)] replicated over w, optionally pre-folded with
     W2 -- pure replication / small-weight fold done on host)
  2. h = VT*QT (one DVE fp16 2x tensor_tensor; w2rep variant adds one more)
  3. t = tanh(h) in place on ScalarE
  4. scores via custom DVE op ANT_MUL_SSCAN: global inclusive cumsum of
     t*w3rep along the stream; per-row sums are differences of consecutive
     row-end elements (one e-segment per column)
  5. softmax over w WITHOUT leaving the layout, using tiny PE matmuls:
     denom[b_sub, col] via a constant [128,6] summing stationary; 1/denom
     on DVE; replicated back to 120 partitions via a constant [6,128]
     stationary; a = e * recrep (small DVE ops)
  6. ablk = a * mask6 gives the block-diagonal moving operand [128, 6] per
     column; context^T accumulates in PSUM via PE matmuls with the VALUE
     TILE ITSELF as the stationary operand (FWL fp16 128x128 weight loads):
     out[e, b_sub] = sum_p VT[p, e] * ablk[p, b_sub]
  7. PSUM -> SBUF drains on ScalarE (fp32->fp16), one fp16 store of ctx^T;
     host transposes back.

Engine budget per core (est): DVE ~300us, ACT ~170us, PE ~115us, DMA ~250us.
"""

import numpy as np

B, W, E = 65536, 20, 128
N_CORES = 8
B_CORE = B // N_CORES

R_CORE = B_CORE * W          # 163840 value rows per core
P = 120                      # real rows per column (6 b's x 20 w's)
NCOL = (R_CORE + P - 1) // P  # 1366 (last col has 40 rows)
NFULL = R_CORE // P          # 1365 full columns
TAIL_ROWS = R_CORE - NFULL * P  # 40
G = 64                       # columns per block
NBLK = (NCOL + G - 1) // G   # 22 (last block has 22 cols)
CTX_COLS = NCOL * 6          # 8196 (>= B_CORE)

# If True, host folds W2 into the replicated query (saves one DVE pass).
QFOLD = True

_CACHE = {}


def _register_scan_op():
    """Custom DVE op: global inclusive cumsum of in0*in1 (fp32 feedback).

    Does NOT reset at subdim row boundaries; callers recover per-row segment
    sums as differences of consecutive row-end elements.
    """
    import re

    import concourse.dve_ops as dops
    from concourse import dve_spec as ds

    for o in dops.OPS:
        if o.name == "ANT_MUL_SSCAN":
            return o

    def _ref(in0, in1, c0, c1, c2):
        x = in0.astype(np.float32) * in1.astype(np.float32)
        return np.cumsum(x, axis=-1)

    spec = ds.Spec(
        body=ds.Scan(ds.AluOp.ADD, ds.Src0 * ds.Src1), reference=_ref
    )
    op = dops.DveOp("ANT_MUL_SSCAN", spec, subdim=True, uops_sha={})
    dops.OPS.append(op)
    dops._SUB_OPCODE_FOR_NAME[op.name] = dops._CUSTOM_DVE_ROW_BASE + len(dops.OPS) - 1
    for ver in ("v3", "v4"):
        try:
            op.compile(ver)
        except ValueError as e:
            m = re.search(r'"([0-9a-f]{16})"', str(e))
            if not m:
                raise
            op.uops_sha[ver] = m.group(1)
            op.compile(ver)
    return op


def _build(
    b_core: int,
    reps: int = 1,
    skip_mm: bool = False,
    skip_scan: bool = False,
    skip_big_dma: bool = False,
    skip_m1: bool = False,
):
    import sys

    if "/opt/trn_rl_repo" not in sys.path:
        sys.path.insert(0, "/opt/trn_rl_repo")
    import concourse.bacc as bacc
    import concourse.bass as bass
    import concourse.mybir as mybir
    import concourse.tile as tile

    assert b_core == B_CORE

    f16 = mybir.dt.float16
    f32 = mybir.dt.float32

    sscan = _register_scan_op()

    nc = bacc.Bacc(
        "TRN2",
        target_bir_lowering=False,
        debug=False,
        enable_asserts=False,
        num_devices=N_CORES,
    )

    value_d = nc.dram_tensor("value", [R_CORE, E], f16, kind="ExternalInput").ap()
    qin_d = nc.dram_tensor("qin", [R_CORE, E], f16, kind="ExternalInput").ap()
    w3rep_d = nc.dram_tensor("w3rep", [128, E], f16, kind="ExternalInput").ap()
    sum6_d = nc.dram_tensor("sum6", [128, 6], f32, kind="ExternalInput").ap()
    rep6_d = nc.dram_tensor("rep6", [6, 128], f32, kind="ExternalInput").ap()
    mask6_d = nc.dram_tensor("mask6", [128, 6], f16, kind="ExternalInput").ap()
    w2rep_d = nc.dram_tensor("w2rep", [128, E], f16, kind="ExternalInput").ap()
    ctxT_d = nc.dram_tensor("ctxT", [E, B_CORE], f16, kind="ExternalOutput").ap()

    mult = mybir.AluOpType.mult
    sub = mybir.AluOpType.subtract
    Tanh = mybir.ActivationFunctionType.Tanh
    Exp = mybir.ActivationFunctionType.Exp

    with tile.TileContext(nc) as tc:
        with (
            tc.tile_pool(name="consts", bufs=1) as cpool,
            tc.tile_pool(name="vbuf", bufs=2) as vpool,
            tc.tile_pool(name="qbuf", bufs=2) as qpool,
            tc.tile_pool(name="csbuf", bufs=1) as cspool,
            tc.tile_pool(name="small", bufs=2) as spool,
            tc.tile_pool(name="ctxps", bufs=2, space="PSUM") as cps,
            tc.tile_pool(name="smps", bufs=2, space="PSUM") as sps,
        ):
            w3t = cpool.tile([128, E], f16, tag="w3t")
            nc.sync.dma_start(w3t[:], w3rep_d)
            sum6 = cpool.tile([128, 6], f32, tag="sum6")
            nc.sync.dma_start(sum6[:], sum6_d)
            rep6 = cpool.tile([6, 128], f32, tag="rep6")
            nc.sync.dma_start(rep6[:], rep6_d)
            mask6 = cpool.tile([128, 6], f16, tag="mask6")
            nc.sync.dma_start(mask6[:], mask6_d)
            w2t = cpool.tile([128, E], f16, tag="w2t")
            nc.sync.dma_start(w2t[:], w2rep_d)
            ctxT = cpool.tile([128, CTX_COLS], f16, tag="ctxT")

            w3b = w3t[:].unsqueeze(1).broadcast_to([128, G, E])
            w2b = w2t[:].unsqueeze(1).broadcast_to([128, G, E])
            m6b = mask6[:].unsqueeze(1).broadcast_to([128, G, 6])

            for it in range(NBLK * reps):
                kb = it % NBLK
                col0 = kb * G
                g = min(G, NCOL - col0)
                gf = g if kb < NBLK - 1 else g - 1  # full columns
                r0 = col0 * P

                VT = vpool.tile([128, G, E], f16)
                QT = qpool.tile([128, G, E], f16)
                if it < 2:
                    # zero the rotating buffers once: partitions 120..127 are
                    # never DMA'd and would feed NaN garbage into the context
                    # matmuls (NaN * 0 = NaN)
                    nc.vector.memset(VT[:], 0.0)
                if kb == NBLK - 1:
                    nc.vector.memset(VT[:, gf, :], 0.0)
                    nc.vector.memset(QT[:, gf, :], 0.0)
                if not skip_big_dma:
                    nc.gpsimd.dma_start(
                        VT[:P, 0:gf, :],
                        value_d[r0 : r0 + P * gf, :].rearrange(
                            "(g p) e -> p g e", p=P
                        ),
                    )
                    nc.gpsimd.dma_start(
                        QT[:P, 0:gf, :],
                        qin_d[r0 : r0 + P * gf, :].rearrange(
                            "(g p) e -> p g e", p=P
                        ),
                    )
                if kb == NBLK - 1:
                    rt = r0 + P * gf
                    nc.gpsimd.dma_start(
                        VT[:TAIL_ROWS, gf, :], value_d[rt : rt + TAIL_ROWS, :]
                    )
                    nc.gpsimd.dma_start(
                        QT[:TAIL_ROWS, gf, :], qin_d[rt : rt + TAIL_ROWS, :]
                    )

                # h = v * qin  (fp16 2x), optional extra *W2 pass
                if not skip_m1:
                    nc.vector.tensor_tensor(
                        QT[:, 0:g, :], QT[:, 0:g, :], VT[:, 0:g, :], mult
                    )
                    if not QFOLD:
                        nc.vector.tensor_tensor(
                            QT[:, 0:g, :], QT[:, 0:g, :], w2b[:, 0:g, :], mult
                        )
                    nc.scalar.activation(QT[:, 0:g, :], QT[:, 0:g, :], Tanh)

                # scores: cumsum(t*w3) along (col, e) stream; row-end diffs
                SC = spool.tile([128, G], f32, tag="sc")
                if not skip_scan:
                    CS = cspool.tile([128, G, E], f32)
                    nc.vector._custom_dve(
                        sscan, out=CS[:, 0:g, :], in0=QT[:, 0:g, :],
                        in1=w3b[:, 0:g, :]
                    )
                    cend = CS[:][:, 0:g, E - 1]  # [128, g]
                    nc.scalar.copy(SC[:, 0:1], cend[:, 0:1])
                    nc.vector.tensor_tensor(
                        SC[:, 1:g], cend[:, 1:g], cend[:, 0 : g - 1], sub
                    )

                E32 = spool.tile([128, G], f32, tag="e32")
                nc.vector.memset(E32[:, 0:g], 0.0)
                nc.scalar.activation(E32[:P, 0:g], SC[:P, 0:g], Exp)

                # softmax over w via PE: denom -> 1/denom -> replicate
                DM = sps.tile([6, G], f32)
                nc.tensor.matmul(DM[:, 0:g], sum6[:], E32[:, 0:g])
                REC = spool.tile([6, G], f32, tag="rec")
                nc.vector.reciprocal(REC[:, 0:g], DM[:, 0:g])
                RR = sps.tile([128, G], f32)
                nc.tensor.matmul(RR[:, 0:g], rep6[:], REC[:, 0:g])
                A = spool.tile([128, G], f16, tag="a")
                nc.vector.tensor_tensor(A[:, 0:g], E32[:, 0:g], RR[:, 0:g], mult)

                # block-diagonal moving operand, then context matmuls
                ABLK = spool.tile([128, G, 6], f16, tag="ablk")
                ab = A[:].unsqueeze(2).broadcast_to([128, G, 6])
                nc.vector.tensor_tensor(
                    ABLK[:, 0:g, :], ab[:, 0:g, :], m6b[:, 0:g, :], mult
                )

                if not skip_mm:
                    CTXP = cps.tile([128, G * 6], f32)
                    for j in range(g):
                        nc.tensor.matmul(
                            CTXP[:, 6 * j : 6 * j + 6],
                            VT[:, j, :],
                            ABLK[:, j, :],
                        )

                    nc.scalar.copy(
                        ctxT[:, 6 * col0 : 6 * (col0 + g)], CTXP[:, 0 : 6 * g]
                    )

            nc.sync.dma_start(ctxT_d, ctxT[:, 0:B_CORE])

    nc.compile()
    return nc


def _get_nc(b_core: int):
    if b_core not in _CACHE:
        _CACHE[b_core] = _build(b_core)
    return _CACHE[b_core]


def _host_prep(query, value, W1, W2, W3):
    """Host-side prep: fp16 casts, query*W1 (tiny) replicated over w, and
    the small constant tensors."""
    q32 = np.asarray(query, dtype=np.float32)
    v32 = np.asarray(value, dtype=np.float32)
    W1 = np.asarray(W1, dtype=np.float32)
    W2 = np.asarray(W2, dtype=np.float32)
    W3 = np.asarray(W3, dtype=np.float32)

    vflat = np.ascontiguousarray(
        v32.reshape(B * W, E), dtype=np.float32
    ).astype(np.float16)

    rq = q32 * W1[0]  # [B, E]
    if QFOLD:
        qin = (rq[:, None, :] * W2[None, :, :]).astype(np.float16)  # [B, W, E]
    else:
        qin = np.broadcast_to(
            rq.astype(np.float16)[:, None, :], (B, W, E)
        )
    qin = np.ascontiguousarray(qin.reshape(B * W, E))

    p = np.arange(128)
    w_of_p = p % W
    live = p < P
    w3rep = np.where(live[:, None], W3[np.minimum(w_of_p, W - 1)], 0.0).astype(
        np.float16
    )
    w2rep = np.where(live[:, None], W2[np.minimum(w_of_p, W - 1)], 0.0).astype(
        np.float16
    )
    bsub = p // W
    sum6 = (
        (bsub[:, None] == np.arange(6)[None, :]) & live[:, None]
    ).astype(np.float32)
    mask6 = sum6.astype(np.float16)
    m = np.arange(128)
    rep6 = (
        ((m[None, :] // W) == np.arange(6)[:, None]) & (m[None, :] < P)
    ).astype(np.float32)

    return vflat, qin, w3rep, w2rep, sum6, rep6, mask6


def make_in_maps(inputs):
    vflat, qin, w3rep, w2rep, sum6, rep6, mask6 = _host_prep(
        inputs["query"], inputs["value"], inputs["W1"], inputs["W2"], inputs["W3"]
    )
    in_maps = []
    for c in range(N_CORES):
        rows = slice(c * R_CORE, (c + 1) * R_CORE)
        in_maps.append(
            {
                "value": np.ascontiguousarray(vflat[rows]),
                "qin": np.ascontiguousarray(qin[rows]),
                "w3rep": w3rep,
                "sum6": sum6,
                "rep6": rep6,
                "mask6": mask6,
                "w2rep": w2rep,
            }
        )
    return in_maps


def kernel(query, value, W1, W2, W3):
    import sys

    if "/opt/trn_rl_repo" not in sys.path:
        sys.path.insert(0, "/opt/trn_rl_repo")
    from concourse.bass_utils import run_bass_kernel_spmd

    inputs = {"query": query, "value": value, "W1": W1, "W2": W2, "W3": W3}
    in_maps = make_in_maps(inputs)
    nc = _get_nc(B_CORE)
    res = run_bass_kernel_spmd(nc, in_maps, list(range(N_CORES)))
    out = np.concatenate(
        [res.results[c]["ctxT"].T for c in range(N_CORES)], axis=0
    )
    return out.astype(np.float32)


# revision 11
# speedup vs baseline: 1.6958x; 1.6958x over previous
"""Bahdanau attention forward on 8 Trainium2 NeuronCores (data-parallel).

Layout: value rows flattened to r = b*W + w, placed at partition p = r % 120
(cols = r // 120), padded to 128 partitions (8 dead rows, masked out).  Each
column j holds the 120 rows of a 6-batch window (b = 6j + p//20, w = p%20).

Per-core pipeline, G=64 columns per block:
  1. fp16 loads: value tile VT and a host-replicated query tensor QT
     (qin[r] = (q*W1)[b(r)] replicated over w, optionally pre-folded with
     W2 -- pure replication / small-weight fold done on host)
  2. h = VT*QT (one DVE fp16 2x tensor_tensor; w2rep variant adds one more)
  3. t = tanh(h) in place on ScalarE
  4. scores via custom DVE op ANT_MUL_SSCAN: global inclusive cumsum of
     t*w3rep along the stream; per-row sums are differences of consecutive
     row-end elements (one e-segment per column)
  5. softmax over w WITHOUT leaving the layout, using tiny PE matmuls:
     denom[b_sub, col] via a constant [128,6] summing stationary; 1/denom
     on DVE; replicated back to 120 partitions via a constant [6,128]
     stationary; a = e * recrep (small DVE ops)
  6. ablk = a * mask6 gives the block-diagonal moving operand [128, 6] per
     column; context^T accumulates in PSUM via PE matmuls with the VALUE
     TILE ITSELF as the stationary operand (FWL fp16 128x128 weight loads):
     out[e, b_sub] = sum_p VT[p, e] * ablk[p, b_sub]
  7. PSUM -> SBUF drains on ScalarE (fp32->fp16), one fp16 store of ctx^T;
     host transposes back.

Engine budget per core (est): DVE ~300us, ACT ~170us, PE ~115us, DMA ~250us.
"""

import numpy as np

B, W, E = 65536, 20, 128
N_CORES = 8
B_CORE = B // N_CORES

R_CORE = B_CORE * W          # 163840 value rows per core
P = 120                      # live partitions (dead: 120..127)
RPP = 2                      # value rows per partition (DMA run = RPP*256B)
BW = 6 * RPP                 # batches per column window
RPC = P * RPP                # value rows per column
NCOL = (R_CORE + RPC - 1) // RPC
NFULL = R_CORE // RPC        # full columns
TAIL_ROWS = R_CORE - NFULL * RPC     # rows in the partial last column
TAIL_PARTS = TAIL_ROWS // RPP        # live partitions in the last column
G = 504 // BW                # columns per block (psum group = 504 fp32 cols)
NBLK = (NCOL + G - 1) // G
CTX_COLS = NCOL * BW         # >= B_CORE

# If True, host folds W2 into the replicated query (saves one DVE pass).
QFOLD = True

_CACHE = {}


def _register_scan_op():
    """Custom DVE op: global inclusive cumsum of in0*in1 (fp32 feedback).

    Does NOT reset at subdim row boundaries; callers recover per-row segment
    sums as differences of consecutive row-end elements.
    """
    import re

    import concourse.dve_ops as dops
    from concourse import dve_spec as ds

    for o in dops.OPS:
        if o.name == "ANT_MUL_SSCAN":
            return o

    def _ref(in0, in1, c0, c1, c2):
        x = in0.astype(np.float32) * in1.astype(np.float32)
        return np.cumsum(x, axis=-1)

    spec = ds.Spec(
        body=ds.Scan(ds.AluOp.ADD, ds.Src0 * ds.Src1), reference=_ref
    )
    op = dops.DveOp("ANT_MUL_SSCAN", spec, subdim=True, uops_sha={})
    dops.OPS.append(op)
    dops._SUB_OPCODE_FOR_NAME[op.name] = dops._CUSTOM_DVE_ROW_BASE + len(dops.OPS) - 1
    for ver in ("v3", "v4"):
        try:
            op.compile(ver)
        except ValueError as e:
            m = re.search(r'"([0-9a-f]{16})"', str(e))
            if not m:
                raise
            op.uops_sha[ver] = m.group(1)
            op.compile(ver)
    return op


def _build(
    b_core: int,
    reps: int = 1,
    skip_mm: bool = False,
    skip_scan: bool = False,
    skip_big_dma: bool = False,
    skip_m1: bool = False,
):
    import sys

    if "/opt/trn_rl_repo" not in sys.path:
        sys.path.insert(0, "/opt/trn_rl_repo")
    import concourse.bacc as bacc
    import concourse.bass as bass
    import concourse.mybir as mybir
    import concourse.tile as tile

    assert b_core == B_CORE

    f16 = mybir.dt.float16
    f32 = mybir.dt.float32

    sscan = _register_scan_op()

    nc = bacc.Bacc(
        "TRN2",
        target_bir_lowering=False,
        debug=False,
        enable_asserts=False,
        num_devices=N_CORES,
    )

    E2 = RPP * E
    value_d = nc.dram_tensor("value", [R_CORE, E], f16, kind="ExternalInput").ap()
    qin_d = nc.dram_tensor("qin", [R_CORE, E], f16, kind="ExternalInput").ap()
    w3rep_d = nc.dram_tensor("w3rep", [128, E2], f16, kind="ExternalInput").ap()
    msum_d = nc.dram_tensor("msum", [128, BW], f32, kind="ExternalInput").ap()
    repm_d = nc.dram_tensor("repm", [BW, 128], f32, kind="ExternalInput").ap()
    maskb_d = nc.dram_tensor("maskb", [128, BW], f16, kind="ExternalInput").ap()
    w2rep_d = nc.dram_tensor("w2rep", [128, E2], f16, kind="ExternalInput").ap()
    ctxT_d = nc.dram_tensor("ctxT", [E, B_CORE], f16, kind="ExternalOutput").ap()

    mult = mybir.AluOpType.mult
    sub = mybir.AluOpType.subtract
    Tanh = mybir.ActivationFunctionType.Tanh
    Exp = mybir.ActivationFunctionType.Exp

    with tile.TileContext(nc) as tc:
        with (
            tc.tile_pool(name="consts", bufs=1) as cpool,
            tc.tile_pool(name="vbuf", bufs=2) as vpool,
            tc.tile_pool(name="qbuf", bufs=2) as qpool,
            tc.tile_pool(name="csbuf", bufs=1) as cspool,
            tc.tile_pool(name="small", bufs=2) as spool,
            tc.tile_pool(name="ctxps", bufs=2, space="PSUM") as cps,
            tc.tile_pool(name="smps", bufs=2, space="PSUM") as sps,
        ):
            w3t = cpool.tile([128, E2], f16, tag="w3t")
            nc.sync.dma_start(w3t[:], w3rep_d)
            msum = cpool.tile([128, BW], f32, tag="msum")
            nc.sync.dma_start(msum[:], msum_d)
            repm = cpool.tile([BW, 128], f32, tag="repm")
            nc.sync.dma_start(repm[:], repm_d)
            maskb = cpool.tile([128, BW], f16, tag="maskb")
            nc.sync.dma_start(maskb[:], maskb_d)
            w2t = cpool.tile([128, E2], f16, tag="w2t")
            nc.sync.dma_start(w2t[:], w2rep_d)
            ctxT = cpool.tile([128, CTX_COLS], f16, tag="ctxT")

            w3b = w3t[:].unsqueeze(1).broadcast_to([128, G, E2])
            w2b = w2t[:].unsqueeze(1).broadcast_to([128, G, E2])
            mbb = (
                maskb[:]
                .unsqueeze(1)
                .unsqueeze(2)
                .broadcast_to([128, G, RPP, BW])
            )

            AXX = mybir.AxisListType.X
            add = mybir.AluOpType.add

            for it in range(NBLK * reps):
                kb = it % NBLK
                col0 = kb * G
                g = min(G, NCOL - col0)
                grpp = g * RPP
                gf = g if kb < NBLK - 1 else g - 1  # full columns
                r0 = col0 * RPC

                VT = vpool.tile([128, G, E2], f16)
                QT = qpool.tile([128, G, E2], f16)
                E32 = spool.tile([128, G * RPP], f32, tag="e32")
                if it < 2:
                    # zero the rotating buffers once: partitions 120..127 are
                    # never DMA'd and would feed NaN garbage into the context
                    # matmuls (NaN * 0 = NaN); same for E32 tails
                    nc.vector.memset(VT[:], 0.0)
                    nc.vector.memset(E32[:], 0.0)
                if kb == NBLK - 1:
                    nc.vector.memset(VT[:, gf, :], 0.0)
                    nc.vector.memset(QT[:, gf, :], 0.0)
                if not skip_big_dma:
                    nc.gpsimd.dma_start(
                        VT[:P, 0:gf, :],
                        value_d[r0 : r0 + RPC * gf, :].rearrange(
                            "(g p k) e -> p g (k e)", p=P, k=RPP
                        ),
                    )
                    nc.gpsimd.dma_start(
                        QT[:P, 0:gf, :],
                        qin_d[r0 : r0 + RPC * gf, :].rearrange(
                            "(g p k) e -> p g (k e)", p=P, k=RPP
                        ),
                    )
                if kb == NBLK - 1:
                    rt = r0 + RPC * gf
                    nc.gpsimd.dma_start(
                        VT[:TAIL_PARTS, gf, :],
                        value_d[rt : rt + TAIL_ROWS, :].rearrange(
                            "(p k) e -> p (k e)", p=TAIL_PARTS
                        ),
                    )
                    nc.gpsimd.dma_start(
                        QT[:TAIL_PARTS, gf, :],
                        qin_d[rt : rt + TAIL_ROWS, :].rearrange(
                            "(p k) e -> p (k e)", p=TAIL_PARTS
                        ),
                    )

                # h = v * qin  (fp16 2x), optional extra *W2 pass
                if not skip_m1:
                    nc.vector.tensor_tensor(
                        QT[:, 0:g, :], QT[:, 0:g, :], VT[:, 0:g, :], mult
                    )
                    if not QFOLD:
                        nc.vector.tensor_tensor(
                            QT[:, 0:g, :], QT[:, 0:g, :], w2b[:, 0:g, :], mult
                        )
                    nc.scalar.activation(QT[:, 0:g, :], QT[:, 0:g, :], Tanh)

                # scores: cumsum(t*w3) along the stream; row-end diffs
                SC = spool.tile([128, G * RPP], f32, tag="sc")
                if not skip_scan:
                    CS = cspool.tile([128, G, E2], f32)
                    nc.vector._custom_dve(
                        sscan, out=CS[:, 0:g, :], in0=QT[:, 0:g, :],
                        in1=w3b[:, 0:g, :]
                    )
                    # row ends: every E-th element of the (col, e) stream
                    csz = CS[:].rearrange("p g (k e) -> p (g k) e", e=E)
                    cend = csz[:, 0:grpp, E - 1]  # [128, grpp], stride E
                    nc.vector.tensor_copy(SC[:, 0:1], cend[:, 0:1])
                    nc.vector.tensor_tensor(
                        SC[:, 1:grpp], cend[:, 1:grpp], cend[:, 0 : grpp - 1],
                        sub,
                    )

                nc.scalar.activation(E32[:P, 0:grpp], SC[:P, 0:grpp], Exp)

                # softmax over w via PE: denom -> 1/denom -> replicate
                E32r = E32[:].rearrange("p (g k) -> p g k", k=RPP)
                ES = spool.tile([128, G], f32, tag="esum")
                nc.vector.tensor_reduce(ES[:, 0:g], E32r[:, 0:g, :], AXX, add)
                DM = sps.tile([BW, G], f32)
                nc.tensor.matmul(DM[:, 0:g], msum[:], ES[:, 0:g])
                REC = spool.tile([BW, G], f32, tag="rec")
                nc.vector.reciprocal(REC[:, 0:g], DM[:, 0:g])
                RR = sps.tile([128, G], f32)
                nc.tensor.matmul(RR[:, 0:g], repm[:], REC[:, 0:g])
                A4 = spool.tile([128, G, RPP], f16, tag="a4")
                rrb = RR[:].unsqueeze(2).broadcast_to([128, G, RPP])
                nc.vector.tensor_tensor(
                    A4[:, 0:g, :], E32r[:, 0:g, :], rrb[:, 0:g, :], mult
                )

                # block-diagonal moving operand, then context matmuls
                ABLK = spool.tile([128, G, RPP, BW], f16, tag="ablk")
                ab = A4[:].unsqueeze(3).broadcast_to([128, G, RPP, BW])
                nc.vector.tensor_tensor(
                    ABLK[:, 0:g, :, :], ab[:, 0:g, :, :], mbb[:, 0:g, :, :], mult
                )

                if not skip_mm:
                    CTXP = cps.tile([128, G * BW], f32)
                    for j in range(g):
                        for k in range(RPP):
                            nc.tensor.matmul(
                                CTXP[:, BW * j : BW * (j + 1)],
                                VT[:, j, k * E : (k + 1) * E],
                                ABLK[:, j, k, :],
                                start=(k == 0),
                                stop=(k == RPP - 1),
                            )

                    nc.scalar.copy(
                        ctxT[:, BW * col0 : BW * (col0 + g)],
                        CTXP[:, 0 : BW * g],
                    )

            if skip_mm:
                nc.vector.memset(ctxT[:], 0.0)
            nc.sync.dma_start(ctxT_d, ctxT[:, 0:B_CORE])

    nc.compile()
    return nc


def _get_nc(b_core: int):
    if b_core not in _CACHE:
        _CACHE[b_core] = _build(b_core)
    return _CACHE[b_core]


def _host_prep(query, value, W1, W2, W3):
    """Host-side prep: fp16 casts, query*W1 (tiny) replicated over w, and
    the small constant tensors."""
    q32 = np.asarray(query, dtype=np.float32)
    v32 = np.asarray(value, dtype=np.float32)
    W1 = np.asarray(W1, dtype=np.float32)
    W2 = np.asarray(W2, dtype=np.float32)
    W3 = np.asarray(W3, dtype=np.float32)

    vflat = np.ascontiguousarray(
        v32.reshape(B * W, E), dtype=np.float32
    ).astype(np.float16)

    rq = q32 * W1[0]  # [B, E]
    if QFOLD:
        qin = (rq[:, None, :] * W2[None, :, :]).astype(np.float16)  # [B, W, E]
    else:
        qin = np.broadcast_to(
            rq.astype(np.float16)[:, None, :], (B, W, E)
        )
    qin = np.ascontiguousarray(qin.reshape(B * W, E))

    p = np.arange(128)
    live = p < P
    # partition p holds rows RPP*p .. RPP*p+RPP-1 (mod RPC) -> w indices
    w_of = (RPP * p[:, None] + np.arange(RPP)[None, :]) % W  # [128, RPP]
    w3rep = np.where(
        live[:, None, None], W3[w_of], 0.0
    ).reshape(128, RPP * E).astype(np.float16)
    w2rep = np.where(
        live[:, None, None], W2[w_of], 0.0
    ).reshape(128, RPP * E).astype(np.float16)
    bsub = (RPP * p) // W  # same for all RPP rows of a partition
    msum = (
        (bsub[:, None] == np.arange(BW)[None, :]) & live[:, None]
    ).astype(np.float32)
    maskb = msum.astype(np.float16)
    m = np.arange(128)
    repm = (
        ((RPP * m[None, :]) // W == np.arange(BW)[:, None]) & (m[None, :] < P)
    ).astype(np.float32)

    return vflat, qin, w3rep, w2rep, msum, repm, maskb


def make_in_maps(inputs):
    vflat, qin, w3rep, w2rep, msum, repm, maskb = _host_prep(
        inputs["query"], inputs["value"], inputs["W1"], inputs["W2"], inputs["W3"]
    )
    in_maps = []
    for c in range(N_CORES):
        rows = slice(c * R_CORE, (c + 1) * R_CORE)
        in_maps.append(
            {
                "value": np.ascontiguousarray(vflat[rows]),
                "qin": np.ascontiguousarray(qin[rows]),
                "w3rep": w3rep,
                "msum": msum,
                "repm": repm,
                "maskb": maskb,
                "w2rep": w2rep,
            }
        )
    return in_maps


def kernel(query, value, W1, W2, W3):
    import sys

    if "/opt/trn_rl_repo" not in sys.path:
        sys.path.insert(0, "/opt/trn_rl_repo")
    from concourse.bass_utils import run_bass_kernel_spmd

    inputs = {"query": query, "value": value, "W1": W1, "W2": W2, "W3": W3}
    in_maps = make_in_maps(inputs)
    nc = _get_nc(B_CORE)
    res = run_bass_kernel_spmd(nc, in_maps, list(range(N_CORES)))
    out = np.concatenate(
        [res.results[c]["ctxT"].T for c in range(N_CORES)], axis=0
    )
    return out.astype(np.float32)
